# revision 41
# baseline (speedup 1.0000x reference)
"""MultiHeadDilatedLocalAttention Trainium2 Bass kernel.

Sharding: data-parallel over batch (2) x tensor-parallel over head groups
(4 groups of 4 heads) = 8 cores. Each core computes q/k/v projections for
its 4 heads (with the two back-to-back linear layers algebraically fused on
host), block-sparse attention exploiting the dilated-local + global-token
mask structure, and a partial output projection. Host sums the 4 partial
output projections per batch.

Mask structure (S=2048, window 257 -> |i-j|<=128, dilation 2, globals 0..31):
scores are computed transposed, ST[tk, tq], per key block kj (16 blocks of
128). Needed tq columns per kj: band strip [128(kj-1), 128(kj+2)) (masked by
the dilated pattern), plus global-q columns [0,32) for kj>=2 (unmasked), plus
for kj=0 the global-k rows tk in [0,32) against all remaining tq (unmasked).
exp() is applied without max-subtraction (scores are O(1) by construction:
weights scaled 0.02, additionally 1/sqrt(dh) folded into Wq). The softmax
denominator is obtained by prepending a ones-column to V so the AV matmul's
output row 0 accumulates sum(P); division happens after AV via
reciprocal + gpsimd partition_broadcast + multiply.
"""

import sys

sys.path.insert(0, "/opt/trn_rl_repo")

import numpy as np

import concourse.bacc as bacc
import concourse.bass as bass
import concourse.mybir as mybir
import concourse.tile as tile

F32 = mybir.dt.float32
F32R = mybir.dt.float32r
BF16 = mybir.dt.bfloat16
AF = mybir.ActivationFunctionType

B, S, D, H = 2, 2048, 1024, 16
DH = D // H  # 64
HG = 4  # heads per core
NCORES = 8
NB = S // 128  # 16 key/query blocks
KD = D // 128  # 8 contraction tiles for projections
GN = 32  # number of global tokens (= N_GLOBAL, asserted at runtime)
WIN = 128  # window // 2
VW = DH + 1  # 65: ones column + head dims


def _strip(kj):
    """Band tq range for key block kj."""
    tq0 = max(0, 128 * (kj - 1))
    tq1 = min(S, 128 * (kj + 2))
    return tq0, tq1 - tq0


def _split512(tq0, n):
    """Split [tq0, tq0+n) at 512 boundaries (PSUM bank limits)."""
    out = []
    while n > 0:
        take = min(n, 512 - (tq0 % 512))
        out.append((tq0, take))
        tq0 += take
        n -= take
    return out


def _pieces(a, b):
    """Split [a, b) at the global-q boundary (32) and 512 bank boundaries."""
    out = []
    cur = a
    while cur < b:
        if cur < GN:
            nxt = min(b, GN)
        else:
            nxt = min(b, 512 * (cur // 512 + 1))
        out.append((cur, nxt - cur))
        cur = nxt
    return out


def build_bass():
    nc = bacc.Bacc()

    xT_d = nc.dram_tensor("xT", [D, S], F32R, kind="ExternalInput")
    wqk_d = nc.dram_tensor("wqkT", [D, 512], F32R, kind="ExternalInput")
    bqk_d = nc.dram_tensor("bqk", [128, 4], F32, kind="ExternalInput")
    wv_d = nc.dram_tensor("wvT", [D, HG * VW], F32R, kind="ExternalInput")
    bv_d = nc.dram_tensor("bvrow", [1, HG * VW], F32R, kind="ExternalInput")
    wo_d = nc.dram_tensor("woT", [HG * DH, D], F32R, kind="ExternalInput")
    mask_d = nc.dram_tensor("maskT", [NB, 128, 384], BF16, kind="ExternalInput")
    ones_d = nc.dram_tensor("ones1", [128, 128], F32R, kind="ExternalInput")
    yT_d = nc.dram_tensor("yT", [D, S], F32, kind="ExternalOutput")

    with tile.TileContext(nc) as tc:
        with (
            tc.tile_pool(name="const", bufs=1) as const,
            tc.tile_pool(name="qkT_p", bufs=1) as qkT_p,
            tc.tile_pool(name="vsb_p", bufs=1) as vsb_p,
            tc.tile_pool(name="avtn_p", bufs=1) as avtn_p,
        ):
            # ---- constants / weights resident in SBUF ----
            # DMA issue order matters: the sync queue drains FIFO, so issue
            # only what phase A needs first; masks/wo/bo2 are issued after
            # the phase-A loop (below) so the first matmuls start early.
            wqk = const.tile([128, KD, 512], F32R)
            wv = const.tile([128, KD, HG * VW], F32R)
            wo = const.tile([128, 2, D], F32R)
            bqk = const.tile([128, 4], F32)
            bvrow = const.tile([1, HG * VW], F32R)
            maskT = const.tile([128, NB, 384], BF16)
            ones1 = const.tile([128, 128], F32R)
            nc.sync.dma_start(bqk[:], bqk_d[:])

            # persistent activations
            qkT = qkT_p.tile([128, 4, S], F32R)  # m=0,1: qT pairs; m=2,3: kT
            v_sb = vsb_p.tile([128, NB, HG * VW], F32R)  # [tok128, block, 4*(1|v)]
            avtn = avtn_p.tile([128, 2, S], F32R)  # normalized AV^T, head pairs

            # ================= Phase A: projections =================
            with (
                tc.tile_pool(name="xa_p", bufs=2) as xa_p,
                tc.tile_pool(name="qk_ps", bufs=4, space="PSUM") as qk_ps,
                tc.tile_pool(name="v_ps", bufs=2, space="PSUM") as v_ps,
            ):
                for n in range(4):  # token strips of 512
                    xa = xa_p.tile([128, KD, 512], F32R, tag="xa")
                    for kt in range(KD):
                        if n == 0:
                            # interleave weight tiles with the first strip so
                            # the first matmul's inputs arrive ASAP
                            nc.sync.dma_start(wqk[:, kt, :],
                                              wqk_d[128 * kt:128 * (kt + 1), :])
                        nc.sync.dma_start(
                            xa[:, kt, :], xT_d[128 * kt:128 * (kt + 1),
                                               512 * n:512 * (n + 1)])
                    if n == 0:
                        for kt in range(KD):
                            nc.sync.dma_start(wv[:, kt, :],
                                              wv_d[128 * kt:128 * (kt + 1), :])
                        nc.sync.dma_start(bvrow[:], bv_d[:])
                        nc.sync.dma_start(ones1[:], ones_d[:])
                    # q/k: psum[dout128, tok512] = wqk_m.T @ x
                    for m in range(4):
                        pqk = qk_ps.tile([128, 512], F32, tag="pqk")
                        for kt in range(KD):
                            nc.tensor.matmul(
                                pqk[:],
                                wqk[:, kt, 128 * m:128 * (m + 1)],
                                xa[:, kt, :],
                                start=(kt == 0), stop=(kt == KD - 1))
                        if m % 2 == 0:
                            nc.vector.tensor_scalar_add(
                                qkT[:, m, 512 * n:512 * (n + 1)],
                                pqk[:], bqk[:, m:m + 1])
                        else:
                            nc.scalar.activation(
                                qkT[:, m, 512 * n:512 * (n + 1)],
                                pqk[:], AF.Identity, bias=bqk[:, m:m + 1])
                    # v: psum[tok128, 260] = x_tb.T @ wv  (+ ones-row bias MM)
                    for tb in range(4):
                        pv = v_ps.tile([128, HG * VW], F32, tag="pv")
                        for kt in range(KD):
                            nc.tensor.matmul(
                                pv[:],
                                xa[:, kt, 128 * tb:128 * (tb + 1)],
                                wv[:, kt, :],
                                start=(kt == 0), stop=False)
                        nc.tensor.matmul(pv[:], ones1[0:1, :], bvrow[:],
                                         start=False, stop=True)
                        nc.vector.tensor_copy(v_sb[:, 4 * n + tb, :], pv[:])
                # masks are first needed by attention; weights wo/bo2 by
                # phase C — keep them behind the x strips in the queue.
                for kj in range(NB):
                    nc.sync.dma_start(maskT[:, kj, :], mask_d[kj, :, :])
                for kt in range(2):
                    nc.sync.dma_start(wo[:, kt, :],
                                      wo_d[128 * kt:128 * (kt + 1), :])

            # ================= Phase B: attention per head =================
            # avt is double-buffered in tq-halves of 1024 so head/half
            # pipelines overlap; global-q columns (tq<32) accumulate in a
            # separate 1-bank avtB across the whole head.
            with (
                tc.tile_pool(name="st_ps", bufs=3, space="PSUM") as st_ps,
                tc.tile_pool(name="avt_ps", bufs=2, space="PSUM") as avt_ps,
                tc.tile_pool(name="avtB_ps", bufs=1, space="PSUM") as avtB_ps,
                tc.tile_pool(name="pt_p", bufs=6) as pt_p,
                tc.tile_pool(name="nrm_p", bufs=3) as nrm_p,
            ):
                def normalize(avt_ap, c0, cn, h, denom_col0):
                    """avtn[., c0:c0+cn] = avt rows 0..64 / denom row 64."""
                    hp = 64 * (h % 2)
                    recip = nrm_p.tile([128, 512], F32R, tag="recip")
                    with nc.allow_low_precision(
                            reason="f32r output is bit-identical to f32"):
                        nc.vector.reciprocal(
                            recip[64:65, 0:cn],
                            avt_ap[64:65, denom_col0:denom_col0 + cn])
                    rb = st_ps.tile([128, 512], F32, tag="st")
                    nc.tensor.matmul(rb[0:64, 0:cn], ones1[64:65, 0:64],
                                     recip[64:65, 0:cn], start=True, stop=True)
                    rb_sb = nrm_p.tile([64, 512], F32, tag="rb_sb")
                    nc.scalar.copy(rb_sb[:, 0:cn], rb[0:64, 0:cn])
                    nc.vector.tensor_mul(
                        avtn[hp:hp + 64, h // 2, c0:c0 + cn],
                        avt_ap[0:64, denom_col0:denom_col0 + cn],
                        rb_sb[:, 0:cn])

                for h in range(HG):
                    hp = 64 * (h % 2)
                    qTh = qkT[hp:hp + 64, h // 2, :]
                    kTh = qkT[hp:hp + 64, 2 + h // 2, :]
                    avtB = avtB_ps.tile([VW, GN], F32, tag="avtB")
                    firstB = [True]
                    for half in (0, 1):
                        lo, hi = 1024 * half, 1024 * (half + 1)
                        avt = avt_ps.tile([VW, 1024], F32, tag="avt")
                        started = set()

                        def avt_acc(vv, pt_ap, base, a, b):
                            for (p0, pn) in _pieces(a, b):
                                rhs = pt_ap[:, p0 - base:p0 - base + pn]
                                if p0 < GN:
                                    nc.tensor.matmul(
                                        avtB[:, p0:p0 + pn], vv, rhs,
                                        start=firstB[0], stop=True,
                                        skip_group_check=not firstB[0])
                                    firstB[0] = False
                                else:
                                    bank = (p0 - lo) // 512
                                    fst = bank not in started
                                    started.add(bank)
                                    nc.tensor.matmul(
                                        avt[:, p0 - lo:p0 - lo + pn], vv, rhs,
                                        start=fst, stop=True,
                                        skip_group_check=not fst)

                        vv0 = v_sb[:, 0, VW * h:VW * (h + 1)]
                        vv0g = v_sb[0:GN, 0, VW * h:VW * (h + 1)]
                        # kj=0 global-k rows (tk<32) vs this half's far tq
                        xchunks = (((256, 512), (512, 1024)) if half == 0
                                   else ((1024, 1536), (1536, 2048)))
                        if half == 0:
                            # kj=0 band [0,256)
                            st = st_ps.tile([128, 512], F32, tag="st")
                            nc.tensor.matmul(st[:, 0:256], kTh[:, 0:128],
                                             qTh[:, 0:256], start=True,
                                             stop=True)
                            pt = pt_p.tile([128, 512], F32R, tag="pt")
                            nc.scalar.activation(pt[:, 0:256], st[:, 0:256],
                                                 AF.Exp)
                            nc.vector.tensor_mul(pt[:, 0:256], pt[:, 0:256],
                                                 maskT[:, 0, 0:256])
                            avt_acc(vv0, pt, 0, 0, 256)
                        for (a, b) in xchunks:
                            stx = st_ps.tile([128, 512], F32, tag="st")
                            nc.tensor.matmul(stx[0:GN, 0:b - a], kTh[:, 0:GN],
                                             qTh[:, a:b], start=True, stop=True)
                            ptx = pt_p.tile([128, 512], F32R, tag="pt")
                            nc.scalar.activation(ptx[0:GN, 0:b - a],
                                                 stx[0:GN, 0:b - a], AF.Exp)
                            avt_acc(vv0g, ptx[0:GN, :], a, a, b)
                        kjs = range(1, 9) if half == 0 else range(7, NB)
                        for kj in kjs:
                            tq0, N = _strip(kj)
                            s0, s1 = max(tq0, lo), min(tq0 + N, hi)
                            M = s1 - s0
                            ctxB = (2 <= kj <= 7) if half == 0 else (kj >= 8)
                            vv = v_sb[:, kj, VW * h:VW * (h + 1)]
                            st = st_ps.tile([128, 512], F32, tag="st")
                            nc.tensor.matmul(st[:, 0:M],
                                             kTh[:, 128 * kj:128 * (kj + 1)],
                                             qTh[:, s0:s1], start=True,
                                             stop=True)
                            if ctxB:
                                nc.tensor.matmul(st[:, 384:384 + GN],
                                                 kTh[:, 128 * kj:128 * (kj + 1)],
                                                 qTh[:, 0:GN], start=True,
                                                 stop=True)
                            pt = pt_p.tile([128, 512], F32R, tag="pt")
                            if ctxB and M == 384:
                                nc.scalar.activation(pt[:, 0:384 + GN],
                                                     st[:, 0:384 + GN], AF.Exp)
                            else:
                                nc.scalar.activation(pt[:, 0:M], st[:, 0:M],
                                                     AF.Exp)
                                if ctxB:
                                    nc.scalar.activation(pt[:, 384:384 + GN],
                                                         st[:, 384:384 + GN],
                                                         AF.Exp)
                            nc.vector.tensor_mul(
                                pt[:, 0:M], pt[:, 0:M],
                                maskT[:, kj, s0 - tq0:s1 - tq0])
                            avt_acc(vv, pt, s0, s0, s1)
                            if ctxB:
                                nc.tensor.matmul(avtB[:, 0:GN], vv,
                                                 pt[:, 384:384 + GN],
                                                 start=False, stop=True,
                                                 skip_group_check=True)
                        # normalize this half's chunks
                        if half == 0:
                            normalize(avt, GN, 512 - GN, h, GN)
                            normalize(avt, 512, 512, h, 512)
                        else:
                            normalize(avt, 1024, 512, h, 0)
                            normalize(avt, 1536, 512, h, 512)
                    # global-q columns
                    normalize(avtB, 0, GN, h, 0)

            # ================= Phase C: output projection =================
            # bo is added host-side after gathering. kt-middle loop so the
            # wo stationary is reused across the 4 token strips; evictions
            # alternate ACT/DVE to balance the vector engines.
            with (
                tc.tile_pool(name="y_ps", bufs=5, space="PSUM") as y_ps,
                tc.tile_pool(name="ye_p", bufs=4) as ye_p,
            ):
                # strip n=0 (tq<512) depends on the final avtB (global-q)
                # normalization of the last head — do it last so the other
                # strips overlap with the phase-B tail
                for m in range(8):
                    yps = {}
                    for n in (1, 2, 3, 0):
                        yp = y_ps.tile([128, 512], F32, tag="yp")
                        yps[n] = yp
                    for kt in range(2):
                        for n in (1, 2, 3, 0):
                            nc.tensor.matmul(yps[n][:],
                                             wo[:, kt, 128 * m:128 * (m + 1)],
                                             avtn[:, kt, 512 * n:512 * (n + 1)],
                                             start=(kt == 0), stop=(kt == 1))
                    for n in (1, 2, 3, 0):
                        ye = ye_p.tile([128, 512], F32, tag="ye")
                        if n % 2 == 0:
                            nc.scalar.copy(ye[:], yps[n][:])
                        else:
                            nc.vector.tensor_copy(ye[:], yps[n][:])
                        nc.sync.dma_start(
                            yT_d[128 * m:128 * (m + 1), 512 * n:512 * (n + 1)],
                            ye[:])
    nc.finalize()
    return nc


def _host_prep(x, wq, bq, wv, bv, pq, pbq, pk, pbk, pv, pbv, wo, bo, global_idx):
    """Fuse the two linear layers, build per-core input maps."""
    f8 = np.float64
    gi = np.asarray(global_idx).astype(np.int64)
    assert np.array_equal(np.sort(gi), np.arange(GN)), (
        "kernel specialized for global_idx == arange(32)")

    scale = 1.0 / np.sqrt(DH)
    Wq = (pq.astype(f8) @ wq.astype(f8)) * scale
    bqe = (pq.astype(f8) @ bq.astype(f8) + pbq.astype(f8)) * scale
    Wk = pk.astype(f8) @ wv.astype(f8)
    bke = pk.astype(f8) @ bv.astype(f8) + pbk.astype(f8)
    Wv = pv.astype(f8) @ wv.astype(f8)
    bve = pv.astype(f8) @ bv.astype(f8) + pbv.astype(f8)

    # mask (from the actual global_idx)
    i = np.arange(S)
    d = i[:, None] - i[None, :]
    g = np.zeros(S, dtype=bool)
    g[gi] = True
    mask = (np.abs(d) <= WIN) & (d % 2 == 0)
    mask |= g[:, None] | g[None, :]

    maskT = np.zeros((NB, 128, 384), dtype=np.float32)
    for kj in range(NB):
        tq0, N = _strip(kj)
        maskT[kj, :, :N] = mask[tq0:tq0 + N, 128 * kj:128 * (kj + 1)].T.astype(
            np.float32)
    import ml_dtypes
    maskT_bf = maskT.astype(ml_dtypes.bfloat16)

    in_maps = []
    for c in range(NCORES):
        b, grp = divmod(c, 4)
        sl = slice(256 * grp, 256 * (grp + 1))

        wqkT = np.concatenate([Wq[sl].T, Wk[sl].T], axis=1)  # [1024, 512]
        bqk2 = np.stack([bqe[sl][0:128], bqe[sl][128:256],
                         bke[sl][0:128], bke[sl][128:256]], axis=1)  # [128,4]

        wvT = np.zeros((D, HG * VW), dtype=f8)
        bvrow = np.zeros((1, HG * VW), dtype=f8)
        for h in range(HG):
            hs = slice(256 * grp + DH * h, 256 * grp + DH * (h + 1))
            wvT[:, VW * h:VW * h + DH] = Wv[hs].T
            bvrow[0, VW * h:VW * h + DH] = bve[hs]
            bvrow[0, VW * h + DH] = 1.0  # ones column last -> denom row 64

        woT = wo[:, sl].T  # [256, 1024]

        in_maps.append({
            "xT": np.ascontiguousarray(x[b].T).astype(np.float32),
            "wqkT": np.ascontiguousarray(wqkT).astype(np.float32),
            "bqk": np.ascontiguousarray(bqk2).astype(np.float32),
            "wvT": np.ascontiguousarray(wvT).astype(np.float32),
            "bvrow": np.ascontiguousarray(bvrow).astype(np.float32),
            "woT": np.ascontiguousarray(woT).astype(np.float32),
            "maskT": maskT_bf,
            "ones1": np.ones((128, 128), np.float32),
        })
    return in_maps


_NC_CACHE = {}


def _get_nc():
    if "nc" not in _NC_CACHE:
        _NC_CACHE["nc"] = build_bass()
    return _NC_CACHE["nc"]


def kernel(**inputs):
    from concourse.bass_utils import run_bass_kernel_spmd

    in_maps = _host_prep(**inputs)
    nc = _get_nc()
    res = run_bass_kernel_spmd(nc, in_maps, core_ids=list(range(NCORES)))
    y = np.zeros((B, S, D), dtype=np.float32)
    for c in range(NCORES):
        b = c // 4
        y[b] += res.results[c]["yT"].T
    y += np.asarray(inputs["bo"], np.float32)  # out-proj bias, added once
    return y


if __name__ == "__main__":
    import reference

    inp = reference.setup_inputs()
    inp = {k: np.asarray(v) for k, v in inp.items()}
    y = kernel(**inp)
    print(y.shape, y.dtype)
